# revision 1
# baseline (speedup 1.0000x reference)
"""ConvTransformerEncoderLayer on 8 trn2 NeuronCores.

Sharding: pure data-parallel over batch (B=8 -> 1 batch element per core).
Each core runs the full layer for its batch element; no collectives.

v4 layout strategy (S=1024, D=512, H=8, hd=64, DFF=2048):
  - all matmuls float32r (self-loading weights: no Ldweights split, full PE
    rate at free-dim 512); PSUM accumulates fp32.
  - Q,K convs produce [c, s]; V conv produces V^T [t, c] (+ ones column per
    head) so AV emits av^T [d, s] with the softmax denominator as a psum row.
  - softmax without max-subtraction (scores are O(10), fp32 exp safe).
  - attention software-pipelined: AV of unit i issues after scores of unit
    i+LAG, so exp (Act engine) hides under conv/scores matmuls.
  - LayerNorm normalize is one DVE tensor_scalar: (z-mu)*rstd; gamma/beta
    folded into W1/b1 host-side (device fixups only when nontrivial).
  - bo+Wo@bv folded into residual src host-side; b1+W1@be1 folded into b1;
    no bias matmuls anywhere.
  - all weights prefetch up front on two HWDGE queues; SBUF slots are
    retagged across phases (X->xs, Q->xT, K->y, VTx->srcs, et->hT).
"""
import sys

sys.path.insert(0, "/opt/trn_rl_repo")
import numpy as np

P = 128          # partitions
S = 1024         # sequence
D = 512          # d_model
H = 8            # heads
HD = 64          # head dim
DFF = 2048
KS = 3           # conv kernel size
EPS = 1e-5
NCORES = 8
CT = D // P      # 4 channel tiles
ST = S // P      # 8 sequence tiles
FT = DFF // P    # 16 ff tiles
SH = 512         # matmul free-dim chunk (= psum bank)
LAG = 2          # attention software-pipeline depth (units)

_CACHE = {}


def _build_nc(flags):
    resid_mul, resid_add, out_mul, out_add = flags
    import concourse.tile as tile
    from concourse import bacc, mybir

    f32 = mybir.dt.float32
    f32r = mybir.dt.float32r
    AF = mybir.ActivationFunctionType
    ALU = mybir.AluOpType

    nc = bacc.Bacc("TRN2", target_bir_lowering=False, debug=False,
                   enable_asserts=False, num_devices=NCORES)

    def din(name, shape, dt=f32r):
        return nc.dram_tensor(name, shape, dt, kind="ExternalInput").ap()

    srcT = din("srcT", [P, CT, S])             # src^T tiled: [p, ct, s]
    src_sd = din("src_sd", [P, ST, D], f32)    # (src + bo2) tiled: [p, st, d]
    wq_d = din("wq", [P, CT, KS, D])           # Wq[co, ci, k] -> [ci_p, ci_t, k, co]
    wk_d = din("wk", [P, CT, KS, D])
    wv_d = din("wv", [P, CT, D])
    wo_d = din("wo", [P, CT, D])               # Wo[e, d] -> [d_p, d_t, e]
    w1_d = din("w1", [P, CT, DFF])             # (W1*g1)[f, d] -> [d_p, d_t, f]
    w2_d = din("w2", [P, FT, D])               # W2[e, f] -> [f_p, f_t, e]
    bq_d = din("bq", [P, CT], f32)
    bk_d = din("bk", [P, CT], f32)
    b1_d = din("b1", [P, FT], f32)             # b1 + W1 @ be1
    id_d = din("ident", [P, P])
    ext = {}
    if resid_mul:
        ext["g1r"] = din("g1r", [P, D], f32)
    if resid_add:
        ext["r1r"] = din("r1r", [P, D], f32)   # be1 + b2 broadcast
    if out_mul:
        ext["g2r"] = din("g2r", [P, D], f32)
    if out_add:
        ext["be2r"] = din("be2r", [P, D], f32)

    out_d = nc.dram_tensor("out", [P, ST, D], f32, kind="ExternalOutput").ap()

    with tile.TileContext(nc) as tc:
        with (
            tc.tile_pool(name="big", bufs=1) as big,
            tc.tile_pool(name="etp", bufs=3) as etp,
            tc.tile_pool(name="small", bufs=1) as small,
            tc.tile_pool(name="tmp", bufs=1) as tmp,
            tc.tile_pool(name="tiny", bufs=4) as tiny,
            tc.tile_pool(name="nrm", bufs=1) as nrm,
            tc.tile_pool(name="psp", bufs=8, space="PSUM") as psp,
        ):
            # ---------- small constants (gpsimd queue, tiny) ----------
            identity = small.tile([P, P], f32r, tag="ident")
            nc.gpsimd.dma_start(identity[:], id_d[:])
            bq_t = small.tile([P, CT], f32, tag="bq")
            nc.gpsimd.dma_start(bq_t[:], bq_d[:])
            bk_t = small.tile([P, CT], f32, tag="bk")
            nc.gpsimd.dma_start(bk_t[:], bk_d[:])
            b1_t = small.tile([P, FT], f32, tag="b1")
            nc.gpsimd.dma_start(b1_t[:], b1_d[:])
            ext_t = {}
            for k in ext:
                ext_t[k] = small.tile([P, D], f32, tag=k)
                nc.gpsimd.dma_start(ext_t[k][:], ext[k][:])
            epsv = small.tile([P, 1], f32, tag="eps")
            nc.vector.memset(epsv[:], EPS)

            # ---------- bulk DMAs: startup-critical first ----------
            X = big.tile([P, CT, S + 2], f32r, tag="A", name="X")
            nc.vector.memset(X[:, :, 0:1].bitcast(f32), 0.0)
            nc.vector.memset(X[:, :, S + 1:S + 2].bitcast(f32), 0.0)
            wv = big.tile([P, CT, D], f32r, tag="WC", name="wv_s")
            nc.scalar.dma_start(wv[:], wv_d[:])
            for ct in range(CT):
                nc.sync.dma_start(X[:, ct, 1:S + 1], srcT[:, ct, :])
            wq = big.tile([P, CT, KS, D], f32r, tag="WA", name="wq_s")
            nc.sync.dma_start(wq[:], wq_d[:])
            wk = big.tile([P, CT, KS, D], f32r, tag="WB", name="wk_s")
            nc.scalar.dma_start(wk[:], wk_d[:])

            Q = big.tile([P, CT, S], f32r, tag="Q", name="Q")
            K = big.tile([P, CT, S], f32r, tag="K", name="K")
            VTx = big.tile([P, ST, H, HD + 1], f32r, tag="V", name="VTx")
            AVT = big.tile([P, CT, S], f32r, tag="AVT", name="AVT")

            # ---------- V conv -> VTx (V^T with a ones column per head) -----
            nc.vector.memset(VTx[:, :, :, HD:HD + 1].bitcast(f32), 1.0)
            for tt in range(ST):
                ps = psp.tile([P, SH], f32, tag="ps", bufs=6, name="psv")
                for ci in range(CT):
                    nc.tensor.matmul(ps[:], X[:, ci, 1 + tt * P:1 + (tt + 1) * P],
                                     wv[:, ci, :],
                                     start=(ci == 0), stop=(ci == CT - 1))
                nc.scalar.activation(VTx[:, tt, :, 0:HD],
                                     ps.rearrange("p (h e) -> p h e", h=H),
                                     AF.Copy)

            # wo into wv's slot (wv dead after V conv)
            wo = big.tile([P, CT, D], f32r, tag="WC", name="wo_s")
            nc.sync.dma_start(wo[:], wo_d[:])

            # ---------- Q/K convs ----------
            def conv_qk(dst, w, bias_t, ct):
                for sc in range(2):
                    ps = psp.tile([P, SH], f32, tag="ps", bufs=6, name="psqk")
                    first = True
                    for ci in range(CT):
                        for k in range(KS):
                            nc.tensor.matmul(
                                ps[:], w[:, ci, k, ct * P:(ct + 1) * P],
                                X[:, ci, sc * SH + k: sc * SH + k + SH],
                                start=first, stop=(ci == CT - 1 and k == KS - 1))
                            first = False
                    nc.vector.tensor_scalar_add(
                        dst[:, ct, sc * SH:(sc + 1) * SH], ps[:],
                        bias_t[:, ct:ct + 1])

            # ---------- attention: scores+exp unit, AV unit ----------
            def scores_unit(h, sc):
                base = HD * (h % 2)
                qh = Q[base:base + HD, h // 2, :]
                kh = K[base:base + HD, h // 2, :]
                et = etp.tile([P, ST, SH], f32r, tag="ET", name=f"et{h}_{sc}")
                for tt in range(ST):
                    ps = psp.tile([P, SH], f32, tag="ps", bufs=6, name="pssc")
                    nc.tensor.matmul(ps[:], kh[:, tt * P:(tt + 1) * P],
                                     qh[:, sc * SH:(sc + 1) * SH],
                                     start=True, stop=True)
                    nc.scalar.activation(et[:, tt, :], ps[:], AF.Exp,
                                         bias=0.0, scale=1.0 / HD)
                return et

            def av_unit(h, sc, et):
                avps = psp.tile([P, SH], f32, tag="ps", bufs=6, name="avps")
                for tt in range(ST):
                    nc.tensor.matmul(avps[0:HD + 1, :], VTx[:, tt, h, :],
                                     et[:, tt, :],
                                     start=(tt == 0), stop=(tt == ST - 1))
                rrec = nrm.tile([1, SH], f32r, tag="rrec", name="rrec")
                with nc.allow_low_precision(reason="f32r softmax denom"):
                    nc.vector.reciprocal(rrec[0:1, :], avps[HD:HD + 1, :])
                rrep = nrm.tile([HD, SH], f32r, tag="rrep", name="rrep")
                nc.gpsimd.partition_broadcast(rrep[:], rrec[0:1, :])
                base_o = HD * (h % 2)
                nc.vector.tensor_tensor(
                    out=AVT[base_o:base_o + HD, h // 2, sc * SH:(sc + 1) * SH],
                    in0=avps[0:HD, :], in1=rrep[:], op=ALU.mult)

            # software pipeline: conv ct group, then scores; AV lags by LAG
            pending = []
            for ct in range(CT):
                conv_qk(Q, wq, bq_t, ct)
                conv_qk(K, wk, bk_t, ct)
                for h in (2 * ct, 2 * ct + 1):
                    for sc in range(2):
                        et = scores_unit(h, sc)
                        pending.append((h, sc, et))
                        if len(pending) > LAG:
                            av_unit(*pending.pop(0))
            for u in pending:
                av_unit(*u)

            # FFN weights into the dead conv-weight slots; srcs into VTx's
            w1 = big.tile([P, CT, DFF], f32r, tag="WA", name="w1_s")
            nc.scalar.dma_start(w1[:], w1_d[:])
            w2 = big.tile([P, FT, D], f32r, tag="WB", name="w2_s")
            nc.sync.dma_start(w2[:], w2_d[:])
            srcs = big.tile([P, ST, D], f32, tag="V", name="srcs")
            nc.gpsimd.dma_start(srcs[:], src_sd[:])
            xs = big.tile([P, ST, D], f32r, tag="A", name="xs")   # X's slot
            xT = big.tile([P, CT, S], f32r, tag="Q", name="xT")   # Q's slot
            y = big.tile([P, ST, D], f32, tag="K", name="y")      # K's slot

            def ln_prep(z):
                """-> (mv, rstd) for DVE normalize of z [P, D]."""
                stats = tiny.tile([P, 6], f32, tag="st6", name="st6")
                nc.vector.bn_stats(stats[:], z[:])
                mv = tiny.tile([P, 2], f32, tag="mv", name="mv")
                nc.vector.bn_aggr(mv[:], stats[:])
                sd = tiny.tile([P, 1], f32, tag="sd", name="sd")
                nc.scalar.activation(sd[:], mv[:, 1:2], AF.Sqrt,
                                     bias=epsv[:], scale=1.0)
                rstd = tiny.tile([P, 1], f32, tag="rstd", name="rstd")
                nc.vector.reciprocal(rstd[:], sd[:])
                return mv, rstd

            # ---------- Wo projection + residual + LN1 ----------
            def wo_ln1(st):
                ps = psp.tile([P, SH], f32, tag="ps", bufs=6, name="pswo")
                for dt in range(CT):
                    nc.tensor.matmul(ps[:], AVT[:, dt, st * P:(st + 1) * P],
                                     wo[:, dt, :], start=(dt == 0),
                                     stop=(dt == CT - 1))
                z = tmp.tile([P, D], f32, tag="t1", bufs=1, name="z1")
                nc.vector.tensor_tensor(out=z[:], in0=ps[:],
                                        in1=srcs[:, st, :], op=ALU.add)
                mv, rstd = ln_prep(z)
                nc.vector.tensor_scalar(out=xs[:, st, :], in0=z[:],
                                        scalar1=mv[:, 0:1], scalar2=rstd[:],
                                        op0=ALU.subtract, op1=ALU.mult)

            def transpose_tile(st, dt):
                tp = psp.tile([P, P], f32r, tag="tp", bufs=2, name="tp")
                nc.tensor.transpose(tp[:], xs[:, st, dt * P:(dt + 1) * P],
                                    identity[:])
                if st % 2 == 0:
                    nc.vector.tensor_copy(xT[:, dt, st * P:(st + 1) * P],
                                          tp[:])
                else:
                    nc.scalar.activation(xT[:, dt, st * P:(st + 1) * P],
                                         tp[:], AF.Copy)

            for st in range(ST // 2):
                wo_ln1(st)
            for st in range(ST // 2):
                for dt in range(CT):
                    transpose_tile(st, dt)

            # ---------- FFN, overlapped with second-half Wo/LN1 ----------
            def ffn1_tile(sc, hT, ft):
                ps = psp.tile([P, SH], f32, tag="ps", bufs=6, name="psf1")
                for dt in range(CT):
                    nc.tensor.matmul(ps[:], w1[:, dt, ft * P:(ft + 1) * P],
                                     xT[:, dt, sc * SH:(sc + 1) * SH],
                                     start=(dt == 0), stop=(dt == CT - 1))
                nc.scalar.activation(hT[ft // 8][:, ft % 8, :], ps[:],
                                     AF.Relu, bias=b1_t[:, ft:ft + 1],
                                     scale=1.0)

            def ffn2_tile(sc, hT, j):
                st = sc * (ST // 2) + j
                ps = psp.tile([P, SH], f32, tag="ps", bufs=6, name="psf2")
                for ft in range(FT):
                    nc.tensor.matmul(
                        ps[:], hT[ft // 8][:, ft % 8, j * P:(j + 1) * P],
                        w2[:, ft, :], start=(ft == 0), stop=(ft == FT - 1))
                if resid_mul or resid_add:
                    xr = tmp.tile([P, D], f32, tag="xr", bufs=1, name="xr")
                    cur = xs[:, st, :]
                    if resid_mul:
                        nc.vector.tensor_tensor(out=xr[:], in0=cur,
                                                in1=ext_t["g1r"][:],
                                                op=ALU.mult)
                        cur = xr[:]
                    if resid_add:
                        nc.vector.tensor_tensor(out=xr[:], in0=cur,
                                                in1=ext_t["r1r"][:],
                                                op=ALU.add)
                    resid_ap = xr[:]
                else:
                    resid_ap = xs[:, st, :]
                z = tmp.tile([P, D], f32, tag="t1", bufs=1, name="z2")
                nc.vector.tensor_tensor(out=z[:], in0=ps[:],
                                        in1=resid_ap, op=ALU.add)
                mv, rstd = ln_prep(z)
                if out_mul or out_add:
                    yt = tmp.tile([P, D], f32, tag="t2", bufs=1, name="yt")
                    nc.vector.tensor_scalar(out=yt[:], in0=z[:],
                                            scalar1=mv[:, 0:1],
                                            scalar2=rstd[:],
                                            op0=ALU.subtract, op1=ALU.mult)
                    cur = yt[:]
                    if out_mul:
                        nc.vector.tensor_tensor(out=y[:, st, :], in0=cur,
                                                in1=ext_t["g2r"][:],
                                                op=ALU.mult)
                        cur = y[:, st, :]
                    if out_add:
                        nc.vector.tensor_tensor(out=y[:, st, :], in0=cur,
                                                in1=ext_t["be2r"][:],
                                                op=ALU.add)
                else:
                    nc.vector.tensor_scalar(out=y[:, st, :], in0=z[:],
                                            scalar1=mv[:, 0:1],
                                            scalar2=rstd[:],
                                            op0=ALU.subtract, op1=ALU.mult)
                nc.sync.dma_start(out_d[:, st, :], y[:, st, :])

            hT0 = [etp.tile([P, FT // 2, SH], f32r, tag="ET",
                            name=f"hT0_{i}") for i in range(2)]
            # interleave: second-half Wo/LN1 between first-half FFN1 tiles
            for ft in range(FT):
                ffn1_tile(0, hT0, ft)
                if ft < ST // 2:
                    wo_ln1(ST // 2 + ft)
                elif ft - ST // 2 < ST // 2:
                    st = ST // 2 + (ft - ST // 2)
                    for dt in range(CT):
                        transpose_tile(st, dt)
            hT1 = [etp.tile([P, FT // 2, SH], f32r, tag="ET",
                            name=f"hT1_{i}") for i in range(2)]
            # interleave: second-half FFN1 between first-half FFN2 tiles
            for j in range(ST // 2):
                ffn2_tile(0, hT0, j)
                for k in range(4):
                    ffn1_tile(1, hT1, 4 * j + k)
            for j in range(ST // 2):
                ffn2_tile(1, hT1, j)

    nc.compile()
    return nc


def _prep_inputs(src, Wq, bq, Wk, bk, Wv, bv, Wo, bo, W1, b1, W2, b2,
                 g1, be1, g2, be2):
    f = np.float32

    def ctile(w):  # [co, ci(, k)] conv weight -> [p, ci_t(, k), co]
        wt = np.ascontiguousarray(np.moveaxis(w, 0, -1))  # [ci(,k), co]
        return np.ascontiguousarray(
            wt.reshape(CT, P, *wt.shape[1:]).transpose(1, 0, *range(2, wt.ndim + 1)))

    W1f = (W1 * np.asarray(g1)[None, :]).astype(f)      # fold gamma1
    b1f = (b1 + W1 @ be1).astype(f)                     # fold beta1
    r1 = (be1 + b2).astype(f)                           # residual additive fix
    flags = (not np.allclose(g1, 1.0), not np.allclose(r1, 0.0),
             not np.allclose(g2, 1.0), not np.allclose(be2, 0.0))

    shared = {
        "wq": ctile(Wq).astype(f),                       # [P, CT, KS, D]
        "wk": ctile(Wk).astype(f),
        "wv": ctile(Wv[:, :, 0]).astype(f),              # [P, CT, D]
        "wo": np.ascontiguousarray(
            Wo.T.reshape(CT, P, D).transpose(1, 0, 2)).astype(f),
        "w1": np.ascontiguousarray(
            W1f.T.reshape(CT, P, DFF).transpose(1, 0, 2)).astype(f),
        "w2": np.ascontiguousarray(
            W2.T.reshape(FT, P, D).transpose(1, 0, 2)).astype(f),
        "bq": np.ascontiguousarray(bq.reshape(CT, P).T).astype(f),
        "bk": np.ascontiguousarray(bk.reshape(CT, P).T).astype(f),
        "b1": np.ascontiguousarray(b1f.reshape(FT, P).T).astype(f),
        "ident": np.eye(P, dtype=f),
    }
    if flags[0]:
        shared["g1r"] = np.ascontiguousarray(np.broadcast_to(g1, (P, D))).astype(f)
    if flags[1]:
        shared["r1r"] = np.ascontiguousarray(np.broadcast_to(r1, (P, D))).astype(f)
    if flags[2]:
        shared["g2r"] = np.ascontiguousarray(np.broadcast_to(g2, (P, D))).astype(f)
    if flags[3]:
        shared["be2r"] = np.ascontiguousarray(np.broadcast_to(be2, (P, D))).astype(f)

    bo2 = (bo + Wo @ bv).astype(f)                       # folded into residual
    in_maps = []
    for b in range(NCORES):
        m = dict(shared)
        m["srcT"] = np.ascontiguousarray(
            src[b].T.reshape(CT, P, S).transpose(1, 0, 2)).astype(f)
        m["src_sd"] = np.ascontiguousarray(
            (src[b] + bo2[None, :]).reshape(ST, P, D).transpose(1, 0, 2)).astype(f)
        in_maps.append(m)
    return in_maps, flags


def get_nc(flags=(False, False, False, False)):
    if ("nc", flags) not in _CACHE:
        _CACHE[("nc", flags)] = _build_nc(flags)
    return _CACHE[("nc", flags)]


def kernel(**inputs):
    from concourse.bass_utils import run_bass_kernel_spmd

    in_maps, flags = _prep_inputs(**{k: np.asarray(v) for k, v in inputs.items()})
    nc = get_nc(flags)
    res = run_bass_kernel_spmd(nc, in_maps, core_ids=list(range(NCORES)))
    outs = [r["out"].transpose(1, 0, 2).reshape(S, D) for r in res.results]
    return np.stack(outs).astype(np.float32)



# revision 6
# speedup vs baseline: 2.1860x; 2.1860x over previous
"""ConvTransformerEncoderLayer on 8 trn2 NeuronCores.

Sharding: pure data-parallel over batch (B=8 -> 1 batch element per core).
Each core runs the full layer for its batch element; no collectives.

v5 layout strategy (S=1024, D=512, H=8, hd=64, DFF=2048):
  - ALL inputs merged into ONE dram tensor "blob" [128, 41112] f32 per core
    (per-call operand count 13 -> 2; the PJRT dispatch path pays a
    per-operand cost that dominated the old per-call time).
  - partition_id input dropped (enable_partition_id=False; no collectives).
  - all matmuls float32r (self-loading weights: no Ldweights split, full PE
    rate at free-dim 512); PSUM accumulates fp32.
  - Q,K convs produce [c, s]; V conv produces V^T [t, c] (+ ones column per
    head) so AV emits av^T [d, s] with the softmax denominator as a psum row.
  - scores of a head PAIR (bases 0/64) issue back-to-back as 64x128 row
    tiles (tile T0/T8) -> concurrent on the PE array, ~2x scores throughput.
  - softmax without max-subtraction (scores are O(10), fp32 exp safe).
  - attention software-pipelined: AV of unit i issues after scores of unit
    i+LAG, so exp (Act engine) hides under conv/scores matmuls.
  - V^T psum->sbuf copy on Pool (Act is the attention-phase bottleneck).
  - LayerNorm normalize is one DVE tensor_scalar: (z-mu)*rstd; gamma/beta
    folded into W1/b1 host-side (device fixups only when nontrivial).
  - bo+Wo@bv folded into residual src host-side; b1+W1@be1 folded into b1;
    no bias matmuls anywhere.
  - SBUF slots are retagged across phases (X->xs, Q->xT, K->y, VTx->srcs,
    et->hT).
"""
import sys

sys.path.insert(0, "/opt/trn_rl_repo")
import numpy as np

P = 128          # partitions
S = 1024         # sequence
D = 512          # d_model
H = 8            # heads
HD = 64          # head dim
DFF = 2048
KS = 3           # conv kernel size
EPS = 1e-5
NCORES = 8
CT = D // P      # 4 channel tiles
ST = S // P      # 8 sequence tiles
FT = DFF // P    # 16 ff tiles
SH = 512         # matmul free-dim chunk (= psum bank)
LAG = 2          # attention software-pipeline depth (pair units)

# blob layout: name -> (offset, length) in fp32 elements per partition.
# Startup-critical regions first (DMA issue order follows blob order).
_BLOB_SPEC = [
    ("srcT", CT * S),          # 4096
    ("wv", CT * D),            # 2048
    ("wq", CT * KS * D),       # 6144
    ("wk", CT * KS * D),       # 6144
    ("bq", CT),
    ("bk", CT),
    ("ident", P),
    ("wo", CT * D),            # 2048
    ("src_sd", ST * D),        # 4096
    ("w1", CT * DFF),          # 8192
    ("b1", FT),
    ("w2", FT * D),            # 8192
]
_EXT_NAMES = ["g1r", "r1r", "g2r", "be2r"]  # appended when flags set

_CACHE = {}


def _blob_layout(flags):
    spec = list(_BLOB_SPEC)
    for name, fl in zip(_EXT_NAMES, flags):
        if fl:
            spec.append((name, D))
    off = {}
    pos = 0
    for name, ln in spec:
        off[name] = (pos, ln)
        pos += ln
    return off, pos


def _build_nc(flags):
    resid_mul, resid_add, out_mul, out_add = flags
    import concourse.tile as tile
    from concourse import bacc, mybir

    f32 = mybir.dt.float32
    f32r = mybir.dt.float32r
    AF = mybir.ActivationFunctionType
    ALU = mybir.AluOpType

    nc = bacc.Bacc("TRN2", target_bir_lowering=False, debug=False,
                   enable_asserts=False, num_devices=NCORES,
                   enable_partition_id=False)

    off, total = _blob_layout(flags)
    blob = nc.dram_tensor("blob", [P, total], f32r, kind="ExternalInput").ap()

    def bsl(name, *shape, dt=None):
        o, ln = off[name]
        ap = blob[:, o:o + ln]
        if shape:
            dims = dict(zip("abc", shape))
            pat = " ".join("abc"[:len(shape)]) + " rest"
            ap = ap.rearrange(f"p ({pat}) -> p " + " ".join("abc"[:len(shape)])
                              + " rest", **dims)
        if dt is not None:
            ap = ap.bitcast(dt)
        return ap

    out_d = nc.dram_tensor("out", [P, ST, D], f32, kind="ExternalOutput").ap()

    with tile.TileContext(nc) as tc:
        with (
            tc.tile_pool(name="big", bufs=1) as big,
            tc.tile_pool(name="etp", bufs=3) as etp,
            tc.tile_pool(name="small", bufs=1) as small,
            tc.tile_pool(name="tmp", bufs=1) as tmp,
            tc.tile_pool(name="tiny", bufs=4) as tiny,
            tc.tile_pool(name="nrm", bufs=1) as nrm,
            tc.tile_pool(name="psp", bufs=8, space="PSUM") as psp,
        ):
            # ---------- small constants (gpsimd queue, tiny) ----------
            identity = small.tile([P, P], f32r, tag="ident")
            nc.gpsimd.dma_start(identity[:], bsl("ident"))
            bq_t = small.tile([P, CT], f32, tag="bq")
            nc.gpsimd.dma_start(bq_t[:], bsl("bq", dt=f32))
            bk_t = small.tile([P, CT], f32, tag="bk")
            nc.gpsimd.dma_start(bk_t[:], bsl("bk", dt=f32))
            b1_t = small.tile([P, FT], f32, tag="b1")
            nc.gpsimd.dma_start(b1_t[:], bsl("b1", dt=f32))
            ext_t = {}
            for k, fl in zip(_EXT_NAMES, flags):
                if not fl:
                    continue
                ext_t[k] = small.tile([P, D], f32, tag=k)
                nc.gpsimd.dma_start(ext_t[k][:], bsl(k, dt=f32))
            epsv = small.tile([P, 1], f32, tag="eps")
            nc.vector.memset(epsv[:], EPS)

            # ---------- bulk DMAs: startup-critical first ----------
            X = big.tile([P, CT, S + 2], f32r, tag="A", name="X")
            nc.vector.memset(X[:, :, 0:1].bitcast(f32), 0.0)
            nc.vector.memset(X[:, :, S + 1:S + 2].bitcast(f32), 0.0)
            wv = big.tile([P, CT, D], f32r, tag="WC", name="wv_s")
            nc.scalar.dma_start(wv[:], bsl("wv", CT))
            srcT_v = bsl("srcT", CT)
            for ct in range(CT):
                nc.sync.dma_start(X[:, ct, 1:S + 1], srcT_v[:, ct, :])
            wq = big.tile([P, CT, KS, D], f32r, tag="WA", name="wq_s")
            nc.sync.dma_start(wq[:], bsl("wq", CT, KS))
            wk = big.tile([P, CT, KS, D], f32r, tag="WB", name="wk_s")
            nc.scalar.dma_start(wk[:], bsl("wk", CT, KS))

            Q = big.tile([P, CT, S], f32r, tag="Q", name="Q")
            K = big.tile([P, CT, S], f32r, tag="K", name="K")
            VTx = big.tile([P, ST, H, HD + 1], f32r, tag="V", name="VTx")
            AVT = big.tile([P, CT, S], f32r, tag="AVT", name="AVT")

            # ---------- V conv -> VTx (V^T with a ones column per head) -----
            nc.vector.memset(VTx[:, :, :, HD:HD + 1].bitcast(f32), 1.0)
            for tt in range(ST):
                ps = psp.tile([P, SH], f32, tag="ps", bufs=6, name="psv")
                for ci in range(CT):
                    nc.tensor.matmul(ps[:], X[:, ci, 1 + tt * P:1 + (tt + 1) * P],
                                     wv[:, ci, :],
                                     start=(ci == 0), stop=(ci == CT - 1))
                nc.vector.tensor_copy(VTx[:, tt, :, 0:HD],
                                      ps.rearrange("p (h e) -> p h e", h=H))

            # wo into wv's slot (wv dead after V conv)
            wo = big.tile([P, CT, D], f32r, tag="WC", name="wo_s")
            nc.sync.dma_start(wo[:], bsl("wo", CT))

            # ---------- Q/K convs ----------
            def conv_qk(dst, w, bias_t, ct):
                for sc in range(2):
                    ps = psp.tile([P, SH], f32, tag="ps", bufs=6, name="psqk")
                    first = True
                    for ci in range(CT):
                        for k in range(KS):
                            nc.tensor.matmul(
                                ps[:], w[:, ci, k, ct * P:(ct + 1) * P],
                                X[:, ci, sc * SH + k: sc * SH + k + SH],
                                start=first, stop=(ci == CT - 1 and k == KS - 1))
                            first = False
                    nc.vector.tensor_scalar_add(
                        dst[:, ct, sc * SH:(sc + 1) * SH], ps[:],
                        bias_t[:, ct:ct + 1])

            # ---------- attention: paired scores+exp unit, AV unit ----------
            def scores_pair(ct, sc):
                """Both heads of ct (partition bases 0/64) as adjacent 64x128
                row-tiled matmuls -> concurrent on T0/T8."""
                ets = []
                pss = []
                for i in range(2):
                    base = HD * i
                    et = etp.tile([P, ST, SH], f32r, tag="ET",
                                  name=f"et{2 * ct + i}_{sc}")
                    ets.append(et)
                for tt in range(ST):
                    for i in range(2):
                        base = HD * i
                        ps = psp.tile([P, SH], f32, tag="ps", bufs=6,
                                      name="pssc")
                        nc.tensor.matmul(
                            ps[:], K[base:base + HD, ct, tt * P:(tt + 1) * P],
                            Q[base:base + HD, ct, sc * SH:(sc + 1) * SH],
                            start=True, stop=True)
                        nc.scalar.activation(ets[i][:, tt, :], ps[:], AF.Exp,
                                             bias=0.0, scale=1.0 / HD)
                return ets

            def av_unit(h, sc, et):
                avps = psp.tile([P, SH], f32, tag="ps", bufs=6, name="avps")
                for tt in range(ST):
                    nc.tensor.matmul(avps[0:HD + 1, :], VTx[:, tt, h, :],
                                     et[:, tt, :],
                                     start=(tt == 0), stop=(tt == ST - 1))
                rrec = nrm.tile([1, SH], f32r, tag="rrec", name="rrec")
                with nc.allow_low_precision(reason="f32r softmax denom"):
                    nc.vector.reciprocal(rrec[0:1, :], avps[HD:HD + 1, :])
                rrep = nrm.tile([HD, SH], f32r, tag="rrep", name="rrep")
                nc.gpsimd.partition_broadcast(rrep[:], rrec[0:1, :])
                base_o = HD * (h % 2)
                nc.vector.tensor_tensor(
                    out=AVT[base_o:base_o + HD, h // 2, sc * SH:(sc + 1) * SH],
                    in0=avps[0:HD, :], in1=rrep[:], op=ALU.mult)

            # software pipeline: conv ct group, then paired scores; AV lags.
            # Drain pending to <=1 BEFORE allocating a pair's 2 et tiles so
            # the 3-slot ET pool never creates a circular WAR on the PE queue.
            pending = []
            for ct in range(CT):
                conv_qk(Q, wq, bq_t, ct)
                conv_qk(K, wk, bk_t, ct)
                for sc in range(2):
                    while len(pending) > 1:
                        av_unit(*pending.pop(0))
                    ets = scores_pair(ct, sc)
                    for i in range(2):
                        pending.append((2 * ct + i, sc, ets[i]))
            for u in pending:
                av_unit(*u)

            # FFN weights into the dead conv-weight slots; srcs into VTx's
            w1 = big.tile([P, CT, DFF], f32r, tag="WA", name="w1_s")
            nc.scalar.dma_start(w1[:], bsl("w1", CT))
            w2 = big.tile([P, FT, D], f32r, tag="WB", name="w2_s")
            nc.sync.dma_start(w2[:], bsl("w2", FT))
            srcs = big.tile([P, ST, D], f32, tag="V", name="srcs")
            nc.gpsimd.dma_start(srcs[:], bsl("src_sd", ST, dt=f32))
            xs = big.tile([P, ST, D], f32r, tag="A", name="xs")   # X's slot
            xT = big.tile([P, CT, S], f32r, tag="Q", name="xT")   # Q's slot
            y = big.tile([P, ST, D], f32, tag="K", name="y")      # K's slot

            def ln_prep(z):
                """-> (mv, rstd) for DVE normalize of z [P, D]."""
                stats = tiny.tile([P, 6], f32, tag="st6", name="st6")
                nc.vector.bn_stats(stats[:], z[:])
                mv = tiny.tile([P, 2], f32, tag="mv", name="mv")
                nc.vector.bn_aggr(mv[:], stats[:])
                sd = tiny.tile([P, 1], f32, tag="sd", name="sd")
                nc.scalar.activation(sd[:], mv[:, 1:2], AF.Sqrt,
                                     bias=epsv[:], scale=1.0)
                rstd = tiny.tile([P, 1], f32, tag="rstd", name="rstd")
                nc.vector.reciprocal(rstd[:], sd[:])
                return mv, rstd

            # ---------- Wo projection + residual + LN1 ----------
            def wo_ln1(st):
                ps = psp.tile([P, SH], f32, tag="ps", bufs=6, name="pswo")
                for dt in range(CT):
                    nc.tensor.matmul(ps[:], AVT[:, dt, st * P:(st + 1) * P],
                                     wo[:, dt, :], start=(dt == 0),
                                     stop=(dt == CT - 1))
                z = tmp.tile([P, D], f32, tag="t1", bufs=1, name="z1")
                nc.vector.tensor_tensor(out=z[:], in0=ps[:],
                                        in1=srcs[:, st, :], op=ALU.add)
                mv, rstd = ln_prep(z)
                nc.vector.tensor_scalar(out=xs[:, st, :], in0=z[:],
                                        scalar1=mv[:, 0:1], scalar2=rstd[:],
                                        op0=ALU.subtract, op1=ALU.mult)

            def transpose_tile(st, dt):
                tp = psp.tile([P, P], f32r, tag="tp", bufs=2, name="tp")
                nc.tensor.transpose(tp[:], xs[:, st, dt * P:(dt + 1) * P],
                                    identity[:])
                if st % 2 == 0:
                    nc.vector.tensor_copy(xT[:, dt, st * P:(st + 1) * P],
                                          tp[:])
                else:
                    nc.scalar.activation(xT[:, dt, st * P:(st + 1) * P],
                                         tp[:], AF.Copy)

            for st in range(ST // 2):
                wo_ln1(st)
            for st in range(ST // 2):
                for dt in range(CT):
                    transpose_tile(st, dt)

            # ---------- FFN, overlapped with second-half Wo/LN1 ----------
            def ffn1_tile(sc, hT, ft):
                ps = psp.tile([P, SH], f32, tag="ps", bufs=6, name="psf1")
                for dt in range(CT):
                    nc.tensor.matmul(ps[:], w1[:, dt, ft * P:(ft + 1) * P],
                                     xT[:, dt, sc * SH:(sc + 1) * SH],
                                     start=(dt == 0), stop=(dt == CT - 1))
                nc.scalar.activation(hT[ft // 8][:, ft % 8, :], ps[:],
                                     AF.Relu, bias=b1_t[:, ft:ft + 1],
                                     scale=1.0)

            def ffn2_tile(sc, hT, j):
                st = sc * (ST // 2) + j
                ps = psp.tile([P, SH], f32, tag="ps", bufs=6, name="psf2")
                for ft in range(FT):
                    nc.tensor.matmul(
                        ps[:], hT[ft // 8][:, ft % 8, j * P:(j + 1) * P],
                        w2[:, ft, :], start=(ft == 0), stop=(ft == FT - 1))
                if resid_mul or resid_add:
                    xr = tmp.tile([P, D], f32, tag="xr", bufs=1, name="xr")
                    cur = xs[:, st, :]
                    if resid_mul:
                        nc.vector.tensor_tensor(out=xr[:], in0=cur,
                                                in1=ext_t["g1r"][:],
                                                op=ALU.mult)
                        cur = xr[:]
                    if resid_add:
                        nc.vector.tensor_tensor(out=xr[:], in0=cur,
                                                in1=ext_t["r1r"][:],
                                                op=ALU.add)
                    resid_ap = xr[:]
                else:
                    resid_ap = xs[:, st, :]
                z = tmp.tile([P, D], f32, tag="t1", bufs=1, name="z2")
                nc.vector.tensor_tensor(out=z[:], in0=ps[:],
                                        in1=resid_ap, op=ALU.add)
                mv, rstd = ln_prep(z)
                if out_mul or out_add:
                    yt = tmp.tile([P, D], f32, tag="t2", bufs=1, name="yt")
                    nc.vector.tensor_scalar(out=yt[:], in0=z[:],
                                            scalar1=mv[:, 0:1],
                                            scalar2=rstd[:],
                                            op0=ALU.subtract, op1=ALU.mult)
                    cur = yt[:]
                    if out_mul:
                        nc.vector.tensor_tensor(out=y[:, st, :], in0=cur,
                                                in1=ext_t["g2r"][:],
                                                op=ALU.mult)
                        cur = y[:, st, :]
                    if out_add:
                        nc.vector.tensor_tensor(out=y[:, st, :], in0=cur,
                                                in1=ext_t["be2r"][:],
                                                op=ALU.add)
                else:
                    nc.vector.tensor_scalar(out=y[:, st, :], in0=z[:],
                                            scalar1=mv[:, 0:1],
                                            scalar2=rstd[:],
                                            op0=ALU.subtract, op1=ALU.mult)
                nc.sync.dma_start(out_d[:, st, :], y[:, st, :])

            hT0 = [etp.tile([P, FT // 2, SH], f32r, tag="ET",
                            name=f"hT0_{i}") for i in range(2)]
            # interleave: second-half Wo/LN1 between first-half FFN1 tiles
            for ft in range(FT):
                ffn1_tile(0, hT0, ft)
                if ft < ST // 2:
                    wo_ln1(ST // 2 + ft)
                elif ft - ST // 2 < ST // 2:
                    st = ST // 2 + (ft - ST // 2)
                    for dt in range(CT):
                        transpose_tile(st, dt)
            hT1 = [etp.tile([P, FT // 2, SH], f32r, tag="ET",
                            name=f"hT1_{i}") for i in range(2)]
            # interleave: second-half FFN1 between first-half FFN2 tiles
            for j in range(ST // 2):
                ffn2_tile(0, hT0, j)
                for k in range(4):
                    ffn1_tile(1, hT1, 4 * j + k)
            for j in range(ST // 2):
                ffn2_tile(1, hT1, j)

    nc.compile()
    return nc


def _prep_inputs(src, Wq, bq, Wk, bk, Wv, bv, Wo, bo, W1, b1, W2, b2,
                 g1, be1, g2, be2):
    f = np.float32

    def ctile(w):  # [co, ci(, k)] conv weight -> [p, ci_t(, k), co]
        wt = np.ascontiguousarray(np.moveaxis(w, 0, -1))  # [ci(,k), co]
        return np.ascontiguousarray(
            wt.reshape(CT, P, *wt.shape[1:]).transpose(1, 0, *range(2, wt.ndim + 1)))

    W1f = (W1 * np.asarray(g1)[None, :]).astype(f)      # fold gamma1
    b1f = (b1 + W1 @ be1).astype(f)                     # fold beta1
    r1 = (be1 + b2).astype(f)                           # residual additive fix
    flags = (not np.allclose(g1, 1.0), not np.allclose(r1, 0.0),
             not np.allclose(g2, 1.0), not np.allclose(be2, 0.0))

    pieces = {
        "wq": ctile(Wq).astype(f),                       # [P, CT, KS, D]
        "wk": ctile(Wk).astype(f),
        "wv": ctile(Wv[:, :, 0]).astype(f),              # [P, CT, D]
        "wo": np.ascontiguousarray(
            Wo.T.reshape(CT, P, D).transpose(1, 0, 2)).astype(f),
        "w1": np.ascontiguousarray(
            W1f.T.reshape(CT, P, DFF).transpose(1, 0, 2)).astype(f),
        "w2": np.ascontiguousarray(
            W2.T.reshape(FT, P, D).transpose(1, 0, 2)).astype(f),
        "bq": np.ascontiguousarray(bq.reshape(CT, P).T).astype(f),
        "bk": np.ascontiguousarray(bk.reshape(CT, P).T).astype(f),
        "b1": np.ascontiguousarray(b1f.reshape(FT, P).T).astype(f),
        "ident": np.eye(P, dtype=f),
    }
    if flags[0]:
        pieces["g1r"] = np.ascontiguousarray(np.broadcast_to(g1, (P, D))).astype(f)
    if flags[1]:
        pieces["r1r"] = np.ascontiguousarray(np.broadcast_to(r1, (P, D))).astype(f)
    if flags[2]:
        pieces["g2r"] = np.ascontiguousarray(np.broadcast_to(g2, (P, D))).astype(f)
    if flags[3]:
        pieces["be2r"] = np.ascontiguousarray(np.broadcast_to(be2, (P, D))).astype(f)

    off, total = _blob_layout(flags)
    shared = np.zeros((P, total), f)
    for name, (o, ln) in off.items():
        if name in ("srcT", "src_sd"):
            continue
        shared[:, o:o + ln] = pieces[name].reshape(P, ln)

    bo2 = (bo + Wo @ bv).astype(f)                       # folded into residual
    o_srcT, l_srcT = off["srcT"]
    o_ssd, l_ssd = off["src_sd"]
    in_maps = []
    for b in range(NCORES):
        m = shared.copy()
        m[:, o_srcT:o_srcT + l_srcT] = np.ascontiguousarray(
            src[b].T.reshape(CT, P, S).transpose(1, 0, 2)).astype(f).reshape(P, l_srcT)
        m[:, o_ssd:o_ssd + l_ssd] = np.ascontiguousarray(
            (src[b] + bo2[None, :]).reshape(ST, P, D).transpose(1, 0, 2)
        ).astype(f).reshape(P, l_ssd)
        in_maps.append({"blob": m})
    return in_maps, flags


def get_nc(flags=(False, False, False, False)):
    if ("nc", flags) not in _CACHE:
        _CACHE[("nc", flags)] = _build_nc(flags)
    return _CACHE[("nc", flags)]


def kernel(**inputs):
    from concourse.bass_utils import run_bass_kernel_spmd

    in_maps, flags = _prep_inputs(**{k: np.asarray(v) for k, v in inputs.items()})
    nc = get_nc(flags)
    res = run_bass_kernel_spmd(nc, in_maps, core_ids=list(range(NCORES)))
    outs = [r["out"].transpose(1, 0, 2).reshape(S, D) for r in res.results]
    return np.stack(outs).astype(np.float32)
